# revision 5
# baseline (speedup 1.0000x reference)
"""BFP (block floating point) activation quantization kernel for Trainium2.

Problem: NCHW input [32, 256, 56, 56] f32. Blocks of 8 consecutive channels
share one exponent (at each (n, h, w) position). Per block:
    maxabs = max |x_i|
    p      = 2^floor(log2(maxabs))        (exponent-only part of maxabs)
    s      = p / 4                        (scale; mantissa_bits = 3)
    q_i    = clip(round_half_even(x_i/s), -7, 7) * s   (0 for all-zero blocks)

Strategy (per core; batch dim sharded 4 per core across 8 cores):
  Layout: partition p = (channel_block cb in [0,32), s-quarter s4 in [0,4)),
  free dims = (ch in [0,8), sp chunk of spatial). All DMA bursts are
  contiguous runs of spatial positions.

  Math trick (all exact in fp32, bit-identical to the reference):
    pb   = bits(maxabs) & 0xFF800000          -> p (power of two)
    invp = bits^-1(0x7F000000 - pb)           -> 1/p (power of two, exact)
    r    = x * invp                           (exact: |r| < 2)
    t    = (r + 1.5*2^21) - 1.5*2^21          -> round-half-even to grid 1/4
    w    = clip(t, -1.75, 1.75)               -> clip(round(x/s),-7,7) / 4
    q    = w * p                              (exact)
  Zero blocks: pb = 0 so q = w * 0 = 0.

  Engine placement (per-core totals vs ~71us HBM roofline):
    DVE:    reduce_max(abs) + mul + clip-tensor_scalar (+AND)   ~67us
    ACT:    the two +/-C rounding adds (Copy with imm bias)     ~44us
    GPSIMD: final mul by p + invp int-subtract                  ~61us
    DMA:    12.8 MB in + 12.8 MB out (HWDGE)                    ~71us
"""

import numpy as np

N, C, H, W = 32, 256, 56, 56
NCORES = 8
NPC = N // NCORES        # batches per core
S = H * W                # 3136
BLK = 8
CB = C // BLK            # 32 channel blocks; partition = (n, cb) -> 4*32 = 128
L = 392                  # spatial chunk per iteration (S/L iterations)
NITER = S // L
C2 = 3145728.0           # 1.5 * 2^21: round-to-nearest-grid-1/4 magic constant

_cached = {}


def _build():
    import concourse.bacc as bacc
    import concourse.tile as tile
    import concourse.mybir as mybir

    nc = bacc.Bacc("TRN2", target_bir_lowering=False, debug=False)
    x_d = nc.dram_tensor("x", [NPC, C, S], mybir.dt.float32, kind="ExternalInput").ap()
    q_d = nc.dram_tensor("q", [NPC, C, S], mybir.dt.float32, kind="ExternalOutput").ap()
    # partition p = (n, cb): n and cb are adjacent in memory so they merge
    # into a single DMA dim of [step 25088, count 128] -> 3-dim DMA APs.
    xv = x_d.rearrange("n (cb ch) s -> (n cb) ch s", ch=BLK)
    qv = q_d.rearrange("n (cb ch) s -> (n cb) ch s", ch=BLK)

    f32, i32 = mybir.dt.float32, mybir.dt.int32
    Alu, Act = mybir.AluOpType, mybir.ActivationFunctionType

    with tile.TileContext(nc) as tc:
        with (
            tc.tile_pool(name="big", bufs=3) as big,
            tc.tile_pool(name="small", bufs=3) as small,
            tc.tile_pool(name="consts", bufs=1) as consts,
        ):
            c7f = consts.tile([128, 1], i32)
            nc.vector.memset(c7f[:], 0x7F000000)
            for h in range(NITER):
                    sl = slice(h * L, (h + 1) * L)
                    X = big.tile([128, BLK, L], f32, tag="X")
                    nc.sync.dma_start(X[:], xv[:, :, sl])

                    m = small.tile([128, L], f32, tag="m")
                    nc.vector.tensor_reduce(
                        out=m[:], in_=X[:].rearrange("p ch sp -> p sp ch"),
                        axis=mybir.AxisListType.X, op=Alu.max,
                        apply_absolute_value=True,
                    )
                    pb = small.tile([128, L], i32, tag="pb")
                    nc.vector.tensor_scalar(
                        out=pb[:], in0=m[:].bitcast(i32),
                        scalar1=-8388608,  # 0xFF800000 as int32
                        scalar2=None, op0=Alu.bitwise_and,
                    )
                    invp = small.tile([128, L], i32, tag="invp")
                    nc.gpsimd.tensor_tensor(
                        out=invp[:], in0=c7f[:].broadcast_to([128, L]),
                        in1=pb[:], op=Alu.subtract,
                    )

                    R = big.tile([128, BLK, L], f32, tag="R")
                    nc.vector.tensor_tensor(
                        out=R[:], in0=X[:],
                        in1=invp[:].bitcast(f32).unsqueeze(1).broadcast_to([128, BLK, L]),
                        op=Alu.mult,
                    )
                    T = big.tile([128, BLK, L], f32, tag="T")
                    nc.scalar.activation(out=T[:], in_=R[:], func=Act.Copy, bias=C2, scale=1.0)
                    nc.scalar.activation(out=T[:], in_=T[:], func=Act.Copy, bias=-C2, scale=1.0)
                    nc.vector.tensor_scalar(
                        out=T[:], in0=T[:], scalar1=-1.75, scalar2=1.75,
                        op0=Alu.max, op1=Alu.min,
                    )
                    nc.gpsimd.tensor_tensor(
                        out=R[:], in0=T[:],
                        in1=pb[:].bitcast(f32).unsqueeze(1).broadcast_to([128, BLK, L]),
                        op=Alu.mult,
                    )
                    nc.sync.dma_start(qv[:, :, sl], R[:])
    nc.compile()
    return nc


def get_nc():
    if "nc" not in _cached:
        _cached["nc"] = _build()
    return _cached["nc"]


def kernel(activations, _trace=False):
    from concourse.bass_utils import run_bass_kernel_spmd

    nc = get_nc()
    a = np.ascontiguousarray(activations, dtype=np.float32).reshape(N, C, S)
    in_maps = [{"x": a[i * NPC:(i + 1) * NPC]} for i in range(NCORES)]
    res = run_bass_kernel_spmd(nc, in_maps, core_ids=list(range(NCORES)), trace=_trace)
    out = np.concatenate([r["q"] for r in res.results], axis=0)
    if _trace:
        kernel.last_results = res
    return out.reshape(N, C, H, W)


# revision 22
# speedup vs baseline: 1.0492x; 1.0492x over previous
"""BFP (block floating point) activation quantization kernel for Trainium2.

Problem: NCHW input [32, 256, 56, 56] f32. Blocks of 8 consecutive channels
share one exponent (at each (n, h, w) position). Per block:
    maxabs = max |x_i|
    p      = 2^floor(log2(maxabs))        (exponent-only part of maxabs)
    s      = p / 4                        (scale; mantissa_bits = 3)
    q_i    = clip(round_half_even(x_i/s), -7, 7) * s   (0 for all-zero blocks)

Strategy (per core; batch dim sharded 4 per core across 8 cores):
  Layout: partition p = (channel_block cb in [0,32), s-quarter s4 in [0,4)),
  free dims = (ch in [0,8), sp chunk of spatial). All DMA bursts are
  contiguous runs of spatial positions.

  Math trick (all exact in fp32, bit-identical to the reference):
    pb   = bits(maxabs) & 0xFF800000          -> p (power of two)
    invp = bits^-1(0x7F000000 - pb)           -> 1/p (power of two, exact)
    r    = x * invp                           (exact: |r| < 2)
    t    = (r + 1.5*2^21) - 1.5*2^21          -> round-half-even to grid 1/4
    w    = clip(t, -1.75, 1.75)               -> clip(round(x/s),-7,7) / 4
    q    = w * p                              (exact)
  Zero blocks: pb = 0 so q = w * 0 = 0.

  Engine placement (per-core totals vs ~71us HBM roofline):
    DVE:    reduce_max(abs) + mul + clip-tensor_scalar (+AND)   ~67us
    ACT:    the two +/-C rounding adds (Copy with imm bias)     ~44us
    GPSIMD: final mul by p + invp int-subtract                  ~61us
    DMA:    12.8 MB in + 12.8 MB out (HWDGE)                    ~71us
"""

import numpy as np

N, C, H, W = 32, 256, 56, 56
NCORES = 8
NPC = N // NCORES        # batches per core
S = H * W                # 3136
BLK = 8
CB = C // BLK            # 32 channel blocks; partition = (n, cb) -> 4*32 = 128
L = 196                  # spatial chunk per iteration (S/L iterations)
NITER = S // L
BIG_BUFS = 12            # pipeline depth (tile pool buffers)
OUT_DMA_ON_ACT = False   # issue out-DMAs from the ACT HWDGE ring
OUT_DELAY = 0            # extra nop stages before out-DMA in the ladder
                         # so it never head-of-line-blocks the DMA FIFO
C2 = 3145728.0           # 1.5 * 2^21: round-to-nearest-grid-1/4 magic constant

_cached = {}


def _build():
    import concourse.bacc as bacc
    import concourse.tile as tile
    import concourse.mybir as mybir

    nc = bacc.Bacc("TRN2", target_bir_lowering=False, debug=False)
    x_d = nc.dram_tensor("x", [NPC, C, S], mybir.dt.float32, kind="ExternalInput").ap()
    q_d = nc.dram_tensor("q", [NPC, C, S], mybir.dt.float32, kind="ExternalOutput").ap()
    # partition p = (n, cb): n and cb are adjacent in memory so they merge
    # into a single DMA dim of [step 25088, count 128] -> 3-dim DMA APs.
    xv = x_d.rearrange("n (cb ch) s -> (n cb) ch s", ch=BLK)
    qv = q_d.rearrange("n (cb ch) s -> (n cb) ch s", ch=BLK)

    f32, i32 = mybir.dt.float32, mybir.dt.int32
    Alu, Act = mybir.AluOpType, mybir.ActivationFunctionType

    with tile.TileContext(nc) as tc:
        with (
            tc.tile_pool(name="big", bufs=BIG_BUFS) as big,
            tc.tile_pool(name="small", bufs=BIG_BUFS) as small,
            tc.tile_pool(name="consts", bufs=1) as consts,
        ):
            c7f = consts.tile([128, 1], i32)
            nc.vector.memset(c7f[:], 0x7F000000)
            cmask = consts.tile([128, 1], i32)
            nc.vector.memset(cmask[:], -8388608)  # 0xFF800000

            bcast = [128, BLK, L]
            Xs, ms, pbs, invps = {}, {}, {}, {}

            def sl(h):
                return slice(h * L, (h + 1) * L)

            def st_dma_in(h):
                Xs[h] = big.tile([128, BLK, L], f32, tag="X", name=f"X{h}")
                nc.sync.dma_start(Xs[h][:], xv[:, :, sl(h)])

            def st_reduce(h):
                ms[h] = small.tile([128, L], f32, tag="m", name=f"m{h}")
                nc.vector.tensor_reduce(
                    out=ms[h][:], in_=Xs[h][:].rearrange("p ch sp -> p sp ch"),
                    axis=mybir.AxisListType.X, op=Alu.max,
                    apply_absolute_value=True,
                )

            def st_params(h):
                # int32 bitwise ops only exist on DVE; int32 subtract is fine
                # on Pool (TensorScalarPtr is rejected on Pool, TensorTensor ok)
                pbs[h] = small.tile([128, L], i32, tag="pb", name=f"pb{h}")
                nc.vector.tensor_scalar(
                    out=pbs[h][:], in0=ms[h][:].bitcast(i32),
                    scalar1=-8388608,  # 0xFF800000 as int32
                    scalar2=None, op0=Alu.bitwise_and,
                )
                invps[h] = small.tile([128, L], i32, tag="invp", name=f"invp{h}")
                nc.gpsimd.tensor_tensor(
                    out=invps[h][:], in0=c7f[:].broadcast_to([128, L]),
                    in1=pbs[h][:], op=Alu.subtract,
                )

            def st_mul(h):
                # r = x / p   (exact power-of-two scaling)
                nc.vector.tensor_tensor(
                    out=Xs[h][:], in0=Xs[h][:],
                    in1=invps[h][:].bitcast(f32).unsqueeze(1).broadcast_to(bcast),
                    op=Alu.mult,
                )

            def st_act1(h):
                # t = r + C2  (rounds r to grid 1/4, half-even)
                nc.scalar.activation(out=Xs[h][:], in_=Xs[h][:], func=Act.Copy, bias=C2, scale=1.0)

            def st_act2(h):
                nc.scalar.activation(out=Xs[h][:], in_=Xs[h][:], func=Act.Copy, bias=-C2, scale=1.0)

            def st_clip(h):
                # w = clip(u, +-1.75)
                nc.vector.tensor_scalar(
                    out=Xs[h][:], in0=Xs[h][:], scalar1=-1.75, scalar2=1.75,
                    op0=Alu.max, op1=Alu.min,
                )

            def st_pmul(h):
                # q = w * p   (walrus rejects TensorScalarPtr on Pool, so the
                # min could not be fused here; plain TensorTensor)
                nc.gpsimd.tensor_tensor(
                    out=Xs[h][:], in0=Xs[h][:],
                    in1=pbs[h][:].bitcast(f32).unsqueeze(1).broadcast_to(bcast),
                    op=Alu.mult,
                )

            def st_dma_out(h):
                out_eng = nc.scalar if OUT_DMA_ON_ACT else nc.sync
                out_eng.dma_start(qv[:, :, sl(h)], Xs[h][:])
                del Xs[h], ms[h], pbs[h], invps[h]

            nop = lambda h: None
            stages = [st_dma_in, st_reduce, st_params, st_mul,
                      st_act1, st_act2, st_clip, st_pmul] + \
                     [nop] * OUT_DELAY + [st_dma_out]
            # software-pipelined emission: tick t runs stage s on iteration
            # t-s, so every engine's instruction stream interleaves
            # iterations and an unmet wait never blocks younger ready work.
            for t in range(NITER + len(stages) - 1):
                for s, stage in enumerate(stages):
                    h = t - s
                    if 0 <= h < NITER:
                        stage(h)
    nc.compile()
    return nc


def get_nc():
    if "nc" not in _cached:
        _cached["nc"] = _build()
    return _cached["nc"]


def kernel(activations, _trace=False):
    from concourse.bass_utils import run_bass_kernel_spmd

    nc = get_nc()
    a = np.ascontiguousarray(activations, dtype=np.float32).reshape(N, C, S)
    in_maps = [{"x": a[i * NPC:(i + 1) * NPC]} for i in range(NCORES)]
    res = run_bass_kernel_spmd(nc, in_maps, core_ids=list(range(NCORES)), trace=_trace)
    out = np.concatenate([r["q"] for r in res.results], axis=0)
    if _trace:
        kernel.last_results = res
    return out.reshape(N, C, H, W)
